# revision 3
# baseline (speedup 1.0000x reference)
"""VQ codebook argmax kernel for Trainium2 (8 NeuronCores, SPMD).

Device (per core, x-shard [2048,128] vs full codebook [16384,128]):
  per 128-token row-tile, PE computes 32 score chunks [128,512] via
  float32r matmuls (full PE rate); DVE folds each group of 4 chunks
  (one [128,2048] PSUM superchunk spanning 4 banks) with elementwise max
  into a running[128,2048] array; 8 superchunks per row-tile. Running
  arrays stream back to DRAM (1MB per tile).

Host: per token, top-R columns of its running row -> rescore the 8*R
candidate codes {k*2048 + j_r} exactly in fp32 -> argmax. R=4 makes the
chance that float32r rounding (~6e-3 abs) hides the true winner
negligible (needs 4 independent near-ties within ~1e-2).

Self-contained: hardcodes N=16384, D=128, C=16384, 8 cores.
"""
import time
import numpy as np

import concourse.bass as bass
import concourse.mybir as mybir
from concourse import bass_utils

N_CORES = 8
D = 128
N_TOK = 16384
C = 16384
TOK_PER_CORE = N_TOK // N_CORES          # 2048
N_TILES = TOK_PER_CORE // 128            # 16
SUPW = 2048                              # superchunk width (4 PSUM banks)
N_SUP = C // SUPW                        # 8 superchunks / row-tile
MM_PER_SUP = SUPW // 512                 # 4 matmuls per superchunk
N_CB_DMA = 4
R_RESCUE = 4

_NC_CACHE = {}


def _build(noop=False):
    mm_dtype = mybir.dt.float32r
    nc = bass.Bass("TRN2", target_bir_lowering=False, debug=False)
    f32 = mybir.dt.float32

    xT_d = nc.dram_tensor("xT", (D, TOK_PER_CORE), mm_dtype, kind="ExternalInput")
    cbT_d = nc.dram_tensor("cbT", (D, C), mm_dtype, kind="ExternalInput")
    runs_d = nc.dram_tensor("runs", (N_TILES, 128, SUPW), f32, kind="ExternalOutput")

    n_in_dma = 1 + N_CB_DMA
    CBW = C // N_CB_DMA

    with (
        nc.sbuf_tensor([D, TOK_PER_CORE], mm_dtype) as xT,
        nc.sbuf_tensor([D, C], mm_dtype) as cbT,
        nc.sbuf_tensor([128, SUPW], f32) as running0,
        nc.sbuf_tensor([128, SUPW], f32) as running1,
        nc.psum_tensor([128, SUPW], f32) as ps0,
        nc.psum_tensor([128, SUPW], f32) as ps1,
        nc.semaphore() as dma_sem,
        nc.semaphore() as pe_sem,
        nc.semaphore() as dve_sem,
        nc.semaphore() as ext_sem,
        nc.semaphore() as out_sem,
        nc.Block() as block,
    ):
        psum = [ps0, ps1]
        running = [running0, running1]

        @block.sync
        def _(sync):
            sync.dma_start(xT[:], xT_d[:]).then_inc(dma_sem, 16)
            for s in range(N_CB_DMA):
                sync.dma_start(
                    cbT[:, s * CBW:(s + 1) * CBW],
                    cbT_d[:, s * CBW:(s + 1) * CBW],
                ).then_inc(dma_sem, 16)
            for t in range(N_TILES):
                if not noop:
                    sync.wait_ge(ext_sem, t + 1)
                sync.dma_start(runs_d[t], running[t % 2][:]).then_inc(out_sem, 16)

        if noop:
            return nc

        @block.tensor
        def _(tensor):
            tensor.wait_ge(dma_sem, 16 * n_in_dma)
            for j in range(N_TILES * N_SUP * MM_PER_SUP):
                sc, m = divmod(j, MM_PER_SUP)         # global superchunk, mm-in-sup
                t, k = divmod(sc, N_SUP)              # row-tile, sup-in-tile
                if sc >= 2:
                    tensor.wait_ge(dve_sem, sc - 1)
                nc.tensor.matmul(
                    psum[sc % 2][:, m * 512:(m + 1) * 512],
                    xT[:, t * 128:(t + 1) * 128],
                    cbT[:, (k * SUPW + m * 512):(k * SUPW + (m + 1) * 512)],
                    start=True,
                    stop=True,
                ).then_inc(pe_sem, 1)

        @block.vector
        def _(vector):
            for sc in range(N_TILES * N_SUP):
                t, k = divmod(sc, N_SUP)
                run = running[t % 2]
                vector.wait_ge(pe_sem, MM_PER_SUP * (sc + 1))
                if k == 0:
                    if t >= 2:
                        vector.wait_ge(out_sem, 16 * (t - 1))
                    op = nc.vector.tensor_copy(run[:], psum[sc % 2][:])
                else:
                    op = nc.vector.tensor_tensor(
                        run[:], psum[sc % 2][:], run[:],
                        op=mybir.AluOpType.max,
                    )
                op.then_inc(dve_sem, 1)
                if k == N_SUP - 1:
                    nc.vector.engine_nop().then_inc(ext_sem, 1)

    return nc


def _get_nc(noop=False):
    key = "noop" if noop else "nc"
    if key not in _NC_CACHE:
        _NC_CACHE[key] = _build(noop)
    return _NC_CACHE[key]


def _run_device(in_maps, noop=False):
    nc = _get_nc(noop)
    return bass_utils.run_bass_kernel_spmd(nc, in_maps, core_ids=list(range(N_CORES)))


def kernel(x: np.ndarray, codebook: np.ndarray):
    x = np.ascontiguousarray(x, dtype=np.float32)
    codebook = np.ascontiguousarray(codebook, dtype=np.float32)

    xT = np.ascontiguousarray(x.T)               # [128, 16384]
    cbT = np.ascontiguousarray(codebook.T)       # [128, 16384]

    in_maps = [
        {
            "xT": np.ascontiguousarray(xT[:, c * TOK_PER_CORE:(c + 1) * TOK_PER_CORE]),
            "cbT": cbT,
        }
        for c in range(N_CORES)
    ]

    res = _run_device(in_maps)

    R = R_RESCUE
    indices = np.empty(N_TOK, dtype=np.int64)
    ar = np.arange(N_SUP, dtype=np.int64) * SUPW      # [8]
    for c in range(N_CORES):
        runs = res.results[c]["runs"].reshape(TOK_PER_CORE, SUPW)
        jr = np.argpartition(-runs, R - 1, axis=1)[:, :R]          # [2048, R]
        cand = (jr[:, :, None] + ar[None, None, :]).reshape(TOK_PER_CORE, -1)
        x_shard = x[c * TOK_PER_CORE:(c + 1) * TOK_PER_CORE]
        cb_cand = codebook[cand]                                   # [2048, 8R, 128]
        s = np.matmul(cb_cand, x_shard[:, :, None])[:, :, 0]       # [2048, 8R]
        smax = s.max(axis=1)
        ties = s >= smax[:, None]
        big = np.iinfo(np.int64).max
        idx_min = np.where(ties, cand, big).min(axis=1)
        indices[c * TOK_PER_CORE:(c + 1) * TOK_PER_CORE] = idx_min

    loss = np.float32(0.0)
    return indices.astype(np.int32), loss


def measure_exec_ns(n_rep=6):
    """Wall-clock-delta estimate of device exec time: full kernel minus
    I/O-only variant, min over n_rep runs each (cancels RPC + transfer)."""
    rng = np.random.default_rng(0)
    xT = rng.standard_normal((D, TOK_PER_CORE), dtype=np.float32)
    cbT = rng.standard_normal((D, C), dtype=np.float32)
    in_maps = [{"xT": xT, "cbT": cbT} for _ in range(N_CORES)]
    for noop in (True, False):
        _run_device(in_maps, noop=noop)   # compile + warm
    t_full, t_noop = [], []
    for _ in range(n_rep):
        a = time.perf_counter(); _run_device(in_maps, noop=True); b = time.perf_counter()
        t_noop.append(b - a)
        a = time.perf_counter(); _run_device(in_maps, noop=False); b = time.perf_counter()
        t_full.append(b - a)
    return (min(t_full) - min(t_noop)) * 1e9
